# revision 1
# baseline (speedup 1.0000x reference)
"""CenterLoss kernel for 8 TRN2 NeuronCores.

Computes mean over all points of min distance to any center:
    points:  [B=8, N=4096, D=256] f32
    centers: [B=8, K=1024, D=256] f32
    out = mean_{b,n} min_k ||points[b,n] - centers[b,k]||_2

Sharding: data-parallel over B (one batch element per core). Each core
computes sum_n min_k dist for its batch; host sums the 8 partials and
divides by B*N.

Per-core algorithm (bf16 matmuls, free-dim reduce):
    psum[n,k] = sum_d pT[d,n]*cT[d,k]                    (PE, bf16, 2 MMs/bank)
    ev = bf16(psum); tts = ev - ||c||^2/2                (ACT evac + DVE 2x sub)
    mx[n] = max_k tts[n,k]                               (DVE max-reduce)
    psq[n] = sum_d p[n,d]^2                              (ACT Square+accum)
    dist[n] = sqrt(max(psq[n] - 2*mx[n], 0))             (DVE + ACT sqrt)
    partial = sum_n dist[n]                              (DVE + ones matmul)

Weights (pointsT) are host-packed per 128-column chunk so each [128,128]
stationary tile is one contiguous DMA; first matmul issues ~5us in.
"""

from contextlib import ExitStack

import ml_dtypes
import numpy as np

import concourse.bass as bass
import concourse.mybir as mybir
import concourse.tile as tile
from concourse import bacc
from concourse.bass import ds
from concourse.bass_utils import run_bass_kernel_spmd

B, N, K, D = 8, 4096, 1024, 256
P = 128
NCORES = 8
MCH = N // P  # 32 row-chunks of 128 points
KH = 512      # matmul moving free dim (one PSUM bank)

F32 = mybir.dt.float32
BF16 = mybir.dt.bfloat16
AF = mybir.ActivationFunctionType
ALU = mybir.AluOpType


def _build_kernel(ctx: ExitStack, tc: tile.TileContext, out, ptpack, centersT, pts):
    nc = tc.nc

    const_pool = ctx.enter_context(tc.tile_pool(name="const", bufs=1))
    sb = ctx.enter_context(tc.tile_pool(name="sb", bufs=1))
    wpool = ctx.enter_context(tc.tile_pool(name="wpool", bufs=4))
    psum_main = ctx.enter_context(tc.tile_pool(name="psum_main", bufs=4, space="PSUM"))
    natp = ctx.enter_context(tc.tile_pool(name="natp", bufs=3))

    # --- centers: load, square, csq row --------------------------------
    cT = []  # centersT d-chunks [128, K] bf16
    for d in range(2):
        t = sb.tile([P, K], BF16, name=f"cT{d}", tag=f"cT{d}")
        nc.sync.dma_start(t[:], centersT[ds(d * P, P), :])
        cT.append(t)

    ones_f = const_pool.tile([P, P], F32, name="ones_f", tag="ones_f")
    nc.vector.memset(ones_f[:], 1.0)
    ones = const_pool.tile([P, P], BF16, name="ones", tag="ones")
    nc.scalar.copy(ones[:], ones_f[:])
    onescol = const_pool.tile([P, 1], F32, name="onescol", tag="onescol")
    nc.vector.memset(onescol[:], 1.0)

    sq = []
    for d in range(2):
        s = sb.tile([P, K], BF16, name=f"sq{d}", tag=f"sq{d}")
        nc.scalar.activation(s[:], cT[d][:], AF.Square)
        sq.append(s)
    csq_psum = psum_main.tile([P, K], F32, name="csq_psum", tag="cross")
    for kh in range(K // KH):
        sl = ds(kh * KH, KH)
        nc.tensor.matmul(csq_psum[:, sl], ones[:], sq[0][:, sl], start=True, stop=False)
        nc.tensor.matmul(csq_psum[:, sl], ones[:], sq[1][:, sl], start=False, stop=True)
    # csq/2 replicated over partitions, bf16, for the DVE subtract
    csqh_rep = sb.tile([P, K], BF16, name="csqh_rep", tag="csqh_rep")
    nc.scalar.activation(csqh_rep[:], csq_psum[:], AF.Copy, scale=0.5)

    # --- main loop over 32 point-chunks ----------------------------------
    mx = const_pool.tile([P, MCH], F32, name="mx", tag="mx")
    psq = const_pool.tile([P, MCH], F32, name="psq", tag="psq")

    for m in range(MCH):
        # psq[n] for this chunk via ACT square + row-accumulate
        pt_nat = natp.tile([P, D], F32, name="pt_nat", tag="nat")
        nc.sync.dma_start(pt_nat[:], pts[ds(m * P, P), :])
        sq_scr = natp.tile([P, D], F32, name="sq_scr", tag="sqscr", bufs=2)
        nc.scalar.activation(
            sq_scr[:], pt_nat[:], AF.Square, accum_out=psq[:, ds(m, 1)]
        )

        # stationary weights for this chunk: [d, n] slices, one DMA each
        w = []
        for d in range(2):
            wt = wpool.tile([P, P], BF16, name="w", tag="w")
            nc.gpsimd.dma_start(wt[:], ptpack[m, ds(d * P, P), :])
            w.append(wt)

        # psum[n,k] = p.c  (per 512-wide bank half)
        ps = psum_main.tile([P, K], F32, name="cross", tag="cross")
        for kh in range(K // KH):
            sl = ds(kh * KH, KH)
            nc.tensor.matmul(ps[:, sl], w[0][:], cT[0][:, sl], start=True, stop=False)
            nc.tensor.matmul(ps[:, sl], w[1][:], cT[1][:, sl], start=False, stop=True)

        # evacuate to bf16, subtract csq/2 (DVE 2x mode), then max-reduce
        ev = natp.tile([P, K], BF16, name="ev", tag="ev", bufs=3)
        nc.scalar.copy(ev[:], ps[:])
        tts = natp.tile([P, K], BF16, name="tts", tag="tts", bufs=2)
        nc.vector.tensor_sub(tts[:], ev[:], csqh_rep[:])
        nc.vector.tensor_reduce(mx[:, ds(m, 1)], tts[:], mybir.AxisListType.X, ALU.max)

    # --- epilogue: dist = sqrt(relu(psq - 2*mx)); partial = sum dist ------
    d2 = const_pool.tile([P, MCH], F32, name="d2", tag="d2")
    nc.vector.tensor_scalar(d2[:], mx[:], -2.0, None, op0=ALU.mult)
    d2b = const_pool.tile([P, MCH], F32, name="d2b", tag="d2b")
    nc.vector.tensor_add(d2b[:], d2[:], psq[:])
    d2r = const_pool.tile([P, MCH], F32, name="d2r", tag="d2r")
    nc.vector.tensor_scalar_max(d2r[:], d2b[:], 0.0)
    dist = const_pool.tile([P, MCH], F32, name="dist", tag="dist")
    nc.scalar.activation(dist[:], d2r[:], AF.Sqrt)
    rowsum = const_pool.tile([P, 1], F32, name="rowsum", tag="rowsum")
    nc.vector.tensor_reduce(rowsum[:], dist[:], mybir.AxisListType.X, ALU.add)
    fin = psum_main.tile([1, 1], F32, name="fin", tag="cross", padded_shape=[P, K])
    nc.tensor.matmul(fin[:], rowsum[:], onescol[:], start=True, stop=True)
    out_sb = const_pool.tile([1, 1], F32, name="out_sb", tag="out_sb")
    nc.scalar.copy(out_sb[:], fin[:])
    nc.gpsimd.dma_start(out[:], out_sb[:])


def build():
    nc = bacc.Bacc(
        "TRN2",
        target_bir_lowering=False,
        debug=False,
        enable_asserts=False,
        num_devices=NCORES,
    )
    ptpack = nc.dram_tensor("ptpack", [MCH, D, P], BF16, kind="ExternalInput").ap()
    centersT = nc.dram_tensor("centersT", [D, K], BF16, kind="ExternalInput").ap()
    pts = nc.dram_tensor("pts", [N, D], F32, kind="ExternalInput").ap()
    out = nc.dram_tensor("out", [1, 1], F32, kind="ExternalOutput").ap()
    with tile.TileContext(nc) as tc, ExitStack() as ctx:
        _build_kernel(ctx, tc, out, ptpack, centersT, pts)
    nc.compile()
    return nc


_NC = None


def _make_in_maps(points: np.ndarray, centers: np.ndarray):
    in_maps = []
    for b in range(B):
        ptT = points[b].T.astype(ml_dtypes.bfloat16)         # [D, N]
        ptpack = np.ascontiguousarray(
            ptT.reshape(D, MCH, P).transpose(1, 0, 2)        # [MCH, D, P]
        )
        in_maps.append(
            {
                "ptpack": ptpack,
                "centersT": np.ascontiguousarray(
                    centers[b].T.astype(ml_dtypes.bfloat16)
                ),
                "pts": np.ascontiguousarray(points[b]),
            }
        )
    return in_maps


def kernel(points, centers, **_run_kwargs):
    global _NC
    points = np.asarray(points, dtype=np.float32)
    centers = np.asarray(centers, dtype=np.float32)
    assert points.shape == (B, N, D) and centers.shape == (B, K, D)
    if _NC is None:
        _NC = build()
    res = run_bass_kernel_spmd(
        _NC, _make_in_maps(points, centers), list(range(NCORES)), **_run_kwargs
    )
    total = sum(float(r["out"][0, 0]) for r in res.results)
    return np.array(total / (B * N), dtype=np.float32)


if __name__ == "__main__":
    pts = np.random.RandomState(0).randn(B, N, D).astype(np.float32)
    ctr = np.random.RandomState(1).randn(B, K, D).astype(np.float32)
    print(kernel(pts, ctr))



# revision 5
# speedup vs baseline: 1.5013x; 1.5013x over previous
"""CenterLoss kernel for 8 TRN2 NeuronCores (v3: fp8 DR + PE-bias + 3-engine evac).

Computes mean over all points of min distance to any center:
    points:  [B=8, N=4096, D=256] f32
    centers: [B=8, K=1024, D=256] f32
    out = mean_{b,n} min_k ||points[b,n] - centers[b,k]||_2

Sharding: data-parallel over B (one batch element per core). Each core
computes sum_n min_k dist for its batch; host sums the 8 partials and
divides by B*N.

Per-core algorithm:
    Inputs pre-quantized to fp8e4m3 on host; psq[n]=||p||^2 host-computed in
    f32 from the quantized points. The -||c||^2/2 bias is folded INTO the
    matmul as an extra fp8 DoubleRow accumulation (stationary = ones on
    partition 0 only; moving rows = coarse fp8(-csq/2) + fp8 residual, so
    bias error ~0.25 abs, smaller than bf16 rounding).

    Per 128-point chunk m (32 chunks), PSUM[n,k] = p.c - csq/2 directly:
      bias matmul (start) + main DR matmul (stop) per 512-wide bank.
    Evacuation (the roofline; N*K f32 values) split across 3 engines:
      type1: DVE Max8 straight from PSUM              (~1.2us DVE)
      type2: ACT copy->bf16; DVE TT-max tree + reduce (~1.0us ACT+1.0 DVE)
      type3: ACT copy->bf16; Pool TT-max tree; DVE 128-wide reduce
    Epilogue: dist = sqrt(relu(psq - 2*mx)); partial = sum_n dist.
"""

from contextlib import ExitStack

import ml_dtypes
import numpy as np

import concourse.bass as bass
import concourse.mybir as mybir
import concourse.tile as tile
from concourse import bacc
from concourse.bass import ds
from concourse.bass_utils import run_bass_kernel_spmd

B, N, K, D = 8, 4096, 1024, 256
P = 128
NCORES = 8
MCH = N // P   # 32 row-chunks of 128 points
KH = 512       # psum free width per matmul (one PSUM bank)
WG = 4         # weight DMA groups
MPG = MCH // WG

F32 = mybir.dt.float32
BF16 = mybir.dt.bfloat16
FP8 = mybir.dt.float8e4
AF = mybir.ActivationFunctionType
ALU = mybir.AluOpType
DR = mybir.MatmulPerfMode.DoubleRow

# per-chunk evacuation type: 1=DVE Max8 straight from PSUM (no ACT),
# 2=ACT evacuate to bf16 + DVE STT-max tree (4x mode) + reduce.
# Pool cannot run tensor ops on this compiler (engine check), so the
# N*K f32 PSUM evacuation is split between ACT and DVE only; counts
# chosen to balance ACT (~996ns/chunk) vs DVE (T1 ~1.24us, T2 ~0.79us).
_PAT = [2, 2, 2, 1, 2, 2, 2, 2, 2, 1, 2, 2, 2, 2, 2, 1]
CHUNK_TYPE = [_PAT[m % 16] for m in range(MCH)]


def _build_kernel(ctx: ExitStack, tc: tile.TileContext, out, wall, cpack_d, cbias_d, psqT_d):
    nc = tc.nc

    const_pool = ctx.enter_context(tc.tile_pool(name="const", bufs=1))
    psum_main = ctx.enter_context(tc.tile_pool(name="psum_main", bufs=4, space="PSUM"))
    evp = ctx.enter_context(tc.tile_pool(name="evp", bufs=4))
    trp = ctx.enter_context(tc.tile_pool(name="trp", bufs=2))
    trp_pool = ctx.enter_context(tc.tile_pool(name="trp_pool", bufs=2))

    # --- bulk input loads -------------------------------------------------
    cpack = const_pool.tile([P, 2, K], FP8, name="cpack", tag="cpack")
    nc.sync.dma_start(cpack[:], cpack_d[:])
    cbias = const_pool.tile([P, 2, K], FP8, name="cbias", tag="cbias")
    nc.vector.memset(cbias[:], 0.0)
    nc.sync.dma_start(cbias[0:1, :, :], cbias_d[:])
    psq = const_pool.tile([P, MCH], F32, name="psq", tag="psq")
    nc.gpsimd.dma_start(psq[:], psqT_d[:])

    wt = []
    for g in range(WG):
        w = const_pool.tile([P, MPG, 2, P], FP8, name=f"wt{g}", tag=f"wt{g}")
        eng = nc.sync if g < 2 else nc.gpsimd
        eng.dma_start(w[:], wall[:, ds(g * MPG, MPG), :, :])
        wt.append(w)

    ones_dr = const_pool.tile([P, 2, P], FP8, name="ones_dr", tag="ones_dr")
    nc.vector.memset(ones_dr[:], 0.0)
    nc.vector.memset(ones_dr[0:1, :, :], 1.0)
    onescol = const_pool.tile([P, 1], F32, name="onescol", tag="onescol")
    nc.vector.memset(onescol[:], 1.0)

    # mx8[:, m, 0] holds max_k(p.c - csq/2) for chunk m
    mx8 = const_pool.tile([P, MCH, 8], F32, name="mx8", tag="mx8")

    # --- main loop over 32 point-chunks, PE work grouped by 4 -------------
    for g4 in range(MCH // 4):
        chunks = range(g4 * 4, g4 * 4 + 4)
        pss = []
        for m in chunks:
            ps = psum_main.tile([P, K], F32, name="cross", tag="cross")
            pss.append(ps)
            for kh in range(K // KH):
                sl = ds(kh * KH, KH)
                nc.tensor.matmul(
                    ps[:, sl], ones_dr[:], cbias[:, :, sl],
                    start=True, stop=False, perf_mode=DR,
                )
        for i, m in enumerate(chunks):
            ps = pss[i]
            w = wt[m // MPG][:, m % MPG, :, :]
            for kh in range(K // KH):
                sl = ds(kh * KH, KH)
                nc.tensor.matmul(
                    ps[:, sl], w, cpack[:, :, sl],
                    start=False, stop=True, perf_mode=DR,
                )

        for i, m in enumerate(chunks):
            ps = pss[i]
            if CHUNK_TYPE[m] == 1:
                nc.vector.max(mx8[:, m, :], ps[:])
            else:
                ev = evp.tile([P, K], BF16, name="ev", tag="ev")
                nc.scalar.copy(ev[:], ps[:])
                # pairwise max via scalar_tensor_tensor: (a*1) max b runs in
                # the DVE 4x mode (TT-max would only get 2x)
                t1 = trp.tile([P, 512], BF16, name="t1", tag="t1")
                nc.vector.scalar_tensor_tensor(
                    t1[:], ev[:, ds(0, 512)], 1.0, ev[:, ds(512, 512)],
                    ALU.mult, ALU.max,
                )
                t2 = trp.tile([P, 256], BF16, name="t2", tag="t2")
                nc.vector.scalar_tensor_tensor(
                    t2[:], t1[:, ds(0, 256)], 1.0, t1[:, ds(256, 256)],
                    ALU.mult, ALU.max,
                )
                t3 = trp.tile([P, 128], BF16, name="t3", tag="t3")
                nc.vector.scalar_tensor_tensor(
                    t3[:], t2[:, ds(0, 128)], 1.0, t2[:, ds(128, 128)],
                    ALU.mult, ALU.max,
                )
                nc.vector.tensor_reduce(
                    mx8[:, m, ds(0, 1)], t3[:], mybir.AxisListType.X, ALU.max
                )

    # --- epilogue: dist = sqrt(relu(psq - 2*mx)); partial = sum dist ------
    mxv = mx8[:, :, 0]
    d2 = const_pool.tile([P, MCH], F32, name="d2", tag="d2")
    nc.vector.tensor_scalar(d2[:], mxv, -2.0, None, op0=ALU.mult)
    d2b = const_pool.tile([P, MCH], F32, name="d2b", tag="d2b")
    nc.vector.tensor_add(d2b[:], d2[:], psq[:])
    d2r = const_pool.tile([P, MCH], F32, name="d2r", tag="d2r")
    nc.vector.tensor_scalar_max(d2r[:], d2b[:], 0.0)
    dist = const_pool.tile([P, MCH], F32, name="dist", tag="dist")
    nc.scalar.activation(dist[:], d2r[:], AF.Sqrt)
    rowsum = const_pool.tile([P, 1], F32, name="rowsum", tag="rowsum")
    nc.vector.tensor_reduce(rowsum[:], dist[:], mybir.AxisListType.X, ALU.add)
    fin = psum_main.tile([1, 1], F32, name="fin", tag="cross", padded_shape=[P, K])
    nc.tensor.matmul(fin[:], rowsum[:], onescol[:], start=True, stop=True)
    out_sb = const_pool.tile([1, 1], F32, name="out_sb", tag="out_sb")
    nc.scalar.copy(out_sb[:], fin[:])
    nc.gpsimd.dma_start(out[:], out_sb[:])


def build():
    nc = bacc.Bacc(
        "TRN2",
        target_bir_lowering=False,
        debug=False,
        enable_asserts=False,
        num_devices=NCORES,
    )
    wall = nc.dram_tensor("wall", [P, MCH, 2, P], FP8, kind="ExternalInput").ap()
    cpack_d = nc.dram_tensor("cpack", [P, 2, K], FP8, kind="ExternalInput").ap()
    cbias_d = nc.dram_tensor("cbias", [1, 2, K], FP8, kind="ExternalInput").ap()
    psqT_d = nc.dram_tensor("psqT", [P, MCH], F32, kind="ExternalInput").ap()
    out = nc.dram_tensor("out", [1, 1], F32, kind="ExternalOutput").ap()
    with tile.TileContext(nc) as tc, ExitStack() as ctx:
        _build_kernel(ctx, tc, out, wall, cpack_d, cbias_d, psqT_d)
    nc.compile()
    return nc


_NC = None


def _make_in_maps(points: np.ndarray, centers: np.ndarray):
    in_maps = []
    for b in range(B):
        p8 = points[b].astype(ml_dtypes.float8_e4m3)    # [N, D]
        c8 = centers[b].astype(ml_dtypes.float8_e4m3)   # [K, D]
        pf = p8.astype(np.float32)
        cf = c8.astype(np.float32)
        psq = np.einsum("nd,nd->n", pf, pf)             # [N]
        negcsqh = -0.5 * np.einsum("kd,kd->k", cf, cf)  # [K]
        coarse = negcsqh.astype(ml_dtypes.float8_e4m3)
        resid = (negcsqh - coarse.astype(np.float32)).astype(ml_dtypes.float8_e4m3)
        cbias = np.stack([coarse, resid])[None]         # [1, 2, K]
        # wall[p, m, s, n] = p8[m*128+n, s*128+p]
        wall = np.ascontiguousarray(
            p8.reshape(MCH, P, 2, P).transpose(3, 0, 2, 1)
        )
        # cpack[p, s, k] = c8[k, s*128+p]
        cpack = np.ascontiguousarray(c8.reshape(K, 2, P).transpose(2, 1, 0))
        in_maps.append(
            {
                "wall": wall,
                "cpack": cpack,
                "cbias": np.ascontiguousarray(cbias),
                "psqT": np.ascontiguousarray(psq.reshape(MCH, P).T),
            }
        )
    return in_maps


def kernel(points, centers, **_run_kwargs):
    global _NC
    points = np.asarray(points, dtype=np.float32)
    centers = np.asarray(centers, dtype=np.float32)
    assert points.shape == (B, N, D) and centers.shape == (B, K, D)
    if _NC is None:
        _NC = build()
    res = run_bass_kernel_spmd(
        _NC, _make_in_maps(points, centers), list(range(NCORES)), **_run_kwargs
    )
    total = sum(float(r["out"][0, 0]) for r in res.results)
    return np.array(total / (B * N), dtype=np.float32)


if __name__ == "__main__":
    pts = np.random.RandomState(0).randn(B, N, D).astype(np.float32)
    ctr = np.random.RandomState(1).randn(B, K, D).astype(np.float32)
    print(kernel(pts, ctr))


# revision 7
# speedup vs baseline: 1.6413x; 1.0933x over previous
"""CenterLoss kernel for 8 TRN2 NeuronCores (v4: sorted-csq tree reduce).

Computes mean over all points of min distance to any center:
    points:  [B=8, N=4096, D=256] f32
    centers: [B=8, K=1024, D=256] f32
    out = mean_{b,n} min_k ||points[b,n] - centers[b,k]||_2

Sharding: data-parallel over B (one batch element per core); host sums the
8 partial sums and divides by B*N.

Per-core algorithm (all fp8e4m3, psq/csq host-precomputed from the
quantized values; HW-calibrated op costs in ns):
    Centers are SORTED by ||c||^2 on host and laid out so that the
    pairwise-max tree's stride-128 "blocks" {j, j+128, ..., j+896} hold 8
    consecutive ranks -> nearly-equal csq within a block. The tree then
    max-reduces RAW cross products (TT-max runs at 2 elem/cycle; a fused
    subtract would force 1x), and a per-block midpoint csq/2 is subtracted
    only at the 128-wide level (block csq spread ~1 -> rel err ~1e-3).

    Per pair of 128-point chunks: 4 DR matmuls (256-deep contraction) into
    a [128, 2, 1024] PSUM tile (4 banks); evacuated to bf16 by ACT (copy,
    ~1.9us/pair) or DVE (tensor_copy) to balance engines. Per group of 8
    chunks: one fused DVE tree [128,8,*]: L1-L3 TT-max (2x), TT-sub cbar,
    L4 TT-max, one 3D tensor_reduce -> mx[:, g*8:g*8+8].
    Epilogue: dist = sqrt(relu(psq - 2*mx)); partial = sum_n dist.
"""

from contextlib import ExitStack

import ml_dtypes
import numpy as np

import concourse.bass as bass
import concourse.mybir as mybir
import concourse.tile as tile
from concourse import bacc
from concourse.bass import ds
from concourse.bass_utils import run_bass_kernel_spmd

B, N, K, D = 8, 4096, 1024, 256
P = 128
NCORES = 8
MCH = N // P     # 32 row-chunks of 128 points
NPAIR = MCH // 2  # 16 chunk-pairs
GRP = 8          # chunks per tree group
NGRP = MCH // GRP
WG = 4           # weight DMA groups
MPG = MCH // WG

F32 = mybir.dt.float32
BF16 = mybir.dt.bfloat16
FP8 = mybir.dt.float8e4
AF = mybir.ActivationFunctionType
ALU = mybir.AluOpType
DR = mybir.MatmulPerfMode.DoubleRow

# pairs whose PSUM is evacuated by DVE tensor_copy instead of ACT, to
# balance ACT (~1.97us/pair) against DVE tree work (~6us/group)
DVE_EVAC_PAIRS = frozenset({1, 9})


def _build_kernel(ctx: ExitStack, tc: tile.TileContext, out, wall, cpack_d, cbar8_d, psqT_d):
    nc = tc.nc

    const_pool = ctx.enter_context(tc.tile_pool(name="const", bufs=1))
    psum_main = ctx.enter_context(tc.tile_pool(name="psum_main", bufs=2, space="PSUM"))
    evp = ctx.enter_context(tc.tile_pool(name="evp", bufs=2))
    trp = ctx.enter_context(tc.tile_pool(name="trp", bufs=2))

    # --- bulk input loads -------------------------------------------------
    cpack = const_pool.tile([P, 2, K], FP8, name="cpack", tag="cpack")
    nc.sync.dma_start(cpack[:], cpack_d[:])
    cbar8 = const_pool.tile([P, GRP, P], BF16, name="cbar8", tag="cbar8")
    nc.sync.dma_start(cbar8[:], cbar8_d[:])
    psq = const_pool.tile([P, MCH], F32, name="psq", tag="psq")
    nc.gpsimd.dma_start(psq[:], psqT_d[:])

    wt = []
    for g in range(WG):
        w = const_pool.tile([P, MPG, 2, P], FP8, name=f"wt{g}", tag=f"wt{g}")
        eng = nc.sync if g < 2 else nc.gpsimd
        eng.dma_start(w[:], wall[:, ds(g * MPG, MPG), :, :])
        wt.append(w)

    onescol = const_pool.tile([P, 1], F32, name="onescol", tag="onescol")
    nc.vector.memset(onescol[:], 1.0)

    mx = const_pool.tile([P, MCH], F32, name="mx", tag="mx")

    # --- main loop: 16 chunk-pairs, tree per group of 8 chunks ------------
    ev8 = None
    for t in range(NPAIR):
        g = (2 * t) // GRP
        slot = (2 * t) % GRP
        if slot == 0:
            ev8 = evp.tile([P, GRP, K], BF16, name="ev8", tag="ev8")

        pp = psum_main.tile([P, 2, K], F32, name="pp", tag="pp")
        for c in range(2):
            m = 2 * t + c
            w = wt[m // MPG][:, m % MPG, :, :]
            for kh in range(K // 512):
                nc.tensor.matmul(
                    pp[:, c, ds(kh * 512, 512)], w, cpack[:, :, ds(kh * 512, 512)],
                    start=True, stop=True, perf_mode=DR,
                )

        evslice = ev8[:, ds(slot, 2), :]
        if t in DVE_EVAC_PAIRS:
            nc.vector.tensor_copy(evslice, pp[:])
        else:
            nc.scalar.copy(evslice, pp[:])

        if slot == GRP - 2:
            # fused tree over the full group of 8 chunks
            t1 = trp.tile([P, GRP, 512], BF16, name="t1", tag="t1")
            nc.vector.tensor_max(t1[:], ev8[:, :, ds(0, 512)], ev8[:, :, ds(512, 512)])
            t2 = trp.tile([P, GRP, 256], BF16, name="t2", tag="t2")
            nc.vector.tensor_max(t2[:], t1[:, :, ds(0, 256)], t1[:, :, ds(256, 256)])
            t3 = trp.tile([P, GRP, P], BF16, name="t3", tag="t3")
            nc.vector.tensor_max(t3[:], t2[:, :, ds(0, P)], t2[:, :, ds(P, P)])
            s3 = trp.tile([P, GRP, P], BF16, name="s3", tag="s3")
            nc.vector.tensor_sub(s3[:], t3[:], cbar8[:])
            t4 = trp.tile([P, GRP, 64], BF16, name="t4", tag="t4")
            nc.vector.tensor_max(t4[:], s3[:, :, ds(0, 64)], s3[:, :, ds(64, 64)])
            nc.vector.tensor_reduce(
                mx[:, ds(g * GRP, GRP)], t4[:], mybir.AxisListType.X, ALU.max
            )

    # --- epilogue: dist = sqrt(relu(psq - 2*mx)); partial = sum dist ------
    d2 = const_pool.tile([P, MCH], F32, name="d2", tag="d2")
    nc.vector.tensor_scalar(d2[:], mx[:], -2.0, None, op0=ALU.mult)
    d2b = const_pool.tile([P, MCH], F32, name="d2b", tag="d2b")
    nc.vector.tensor_add(d2b[:], d2[:], psq[:])
    d2r = const_pool.tile([P, MCH], F32, name="d2r", tag="d2r")
    nc.vector.tensor_scalar_max(d2r[:], d2b[:], 0.0)
    dist = const_pool.tile([P, MCH], F32, name="dist", tag="dist")
    nc.scalar.activation(dist[:], d2r[:], AF.Sqrt)
    rowsum = const_pool.tile([P, 1], F32, name="rowsum", tag="rowsum")
    nc.vector.tensor_reduce(rowsum[:], dist[:], mybir.AxisListType.X, ALU.add)
    fin = psum_main.tile([1, 1, 1], F32, name="fin", tag="pp", padded_shape=[P, 2, K])
    nc.tensor.matmul(fin[:], rowsum[:], onescol[:], start=True, stop=True)
    out_sb = const_pool.tile([1, 1], F32, name="out_sb", tag="out_sb")
    nc.scalar.copy(out_sb[:], fin[:])
    nc.gpsimd.dma_start(out[:], out_sb[:])


def build():
    nc = bacc.Bacc(
        "TRN2",
        target_bir_lowering=False,
        debug=False,
        enable_asserts=False,
        num_devices=NCORES,
    )
    wall = nc.dram_tensor("wall", [P, MCH, 2, P], FP8, kind="ExternalInput").ap()
    cpack_d = nc.dram_tensor("cpack", [P, 2, K], FP8, kind="ExternalInput").ap()
    cbar8_d = nc.dram_tensor("cbar8", [P, GRP, P], BF16, kind="ExternalInput").ap()
    psqT_d = nc.dram_tensor("psqT", [P, MCH], F32, kind="ExternalInput").ap()
    out = nc.dram_tensor("out", [1, 1], F32, kind="ExternalOutput").ap()
    with tile.TileContext(nc) as tc, ExitStack() as ctx:
        _build_kernel(ctx, tc, out, wall, cpack_d, cbar8_d, psqT_d)
    nc.compile()
    return nc


_NC = None


def _make_in_maps(points: np.ndarray, centers: np.ndarray):
    # column c of cpack holds sorted-rank r = (c % 128)*8 + c//128, so the
    # max-tree block {j, j+128, ..., j+896} covers ranks 8j..8j+7
    cols = np.arange(K)
    rank_of_col = (cols % P) * GRP + cols // P
    in_maps = []
    for b in range(B):
        p8 = points[b].astype(ml_dtypes.float8_e4m3)    # [N, D]
        c8 = centers[b].astype(ml_dtypes.float8_e4m3)   # [K, D]
        pf = p8.astype(np.float32)
        cf = c8.astype(np.float32)
        psq = np.einsum("nd,nd->n", pf, pf)             # [N]
        csqh = 0.5 * np.einsum("kd,kd->k", cf, cf)      # [K]
        order = np.argsort(csqh)
        c8i = c8[order][rank_of_col]                    # centers in column order
        blk = csqh[order].reshape(P, GRP)               # block j = ranks 8j..8j+7
        cbar = 0.5 * (blk.min(1) + blk.max(1))          # [128] midpoint
        cbar8 = np.broadcast_to(
            cbar.astype(ml_dtypes.bfloat16)[None, None, :], (P, GRP, P)
        )
        # wall[p, m, s, n] = p8[m*128+n, s*128+p]
        wall = np.ascontiguousarray(
            p8.reshape(MCH, P, 2, P).transpose(3, 0, 2, 1)
        )
        # cpack[p, s, k] = c8i[k, s*128+p]
        cpack = np.ascontiguousarray(c8i.reshape(K, 2, P).transpose(2, 1, 0))
        in_maps.append(
            {
                "wall": wall,
                "cpack": cpack,
                "cbar8": np.ascontiguousarray(cbar8),
                "psqT": np.ascontiguousarray(psq.reshape(MCH, P).T),
            }
        )
    return in_maps


def kernel(points, centers, **_run_kwargs):
    global _NC
    points = np.asarray(points, dtype=np.float32)
    centers = np.asarray(centers, dtype=np.float32)
    assert points.shape == (B, N, D) and centers.shape == (B, K, D)
    if _NC is None:
        _NC = build()
    res = run_bass_kernel_spmd(
        _NC, _make_in_maps(points, centers), list(range(NCORES)), **_run_kwargs
    )
    total = sum(float(r["out"][0, 0]) for r in res.results)
    return np.array(total / (B * N), dtype=np.float32)


if __name__ == "__main__":
    pts = np.random.RandomState(0).randn(B, N, D).astype(np.float32)
    ctr = np.random.RandomState(1).randn(B, K, D).astype(np.float32)
    print(kernel(pts, ctr))


# revision 10
# speedup vs baseline: 1.7816x; 1.0855x over previous
"""CenterLoss kernel for 8 TRN2 NeuronCores (v4: sorted-csq tree reduce).

Computes mean over all points of min distance to any center:
    points:  [B=8, N=4096, D=256] f32
    centers: [B=8, K=1024, D=256] f32
    out = mean_{b,n} min_k ||points[b,n] - centers[b,k]||_2

Sharding: data-parallel over B (one batch element per core); host sums the
8 partial sums and divides by B*N.

Per-core algorithm (all fp8e4m3, psq/csq host-precomputed from the
quantized values; HW-calibrated op costs in ns):
    Centers are SORTED by ||c||^2 on host and laid out so that the
    pairwise-max tree's stride-128 "blocks" {j, j+128, ..., j+896} hold 8
    consecutive ranks -> nearly-equal csq within a block. The tree then
    max-reduces RAW cross products (TT-max runs at 2 elem/cycle; a fused
    subtract would force 1x), and a per-block midpoint csq/2 is subtracted
    only at the 128-wide level (block csq spread ~1 -> rel err ~1e-3).

    Per pair of 128-point chunks: 4 DR matmuls (256-deep contraction) into
    a [128, 2, 1024] PSUM tile (4 banks); evacuated to bf16 by ACT (copy,
    ~1.9us/pair) or DVE (tensor_copy) to balance engines. Per group of 8
    chunks: one fused DVE tree [128,8,*]: L1-L3 TT-max (2x), TT-sub cbar,
    L4 TT-max, one 3D tensor_reduce -> mx[:, g*8:g*8+8].
    Epilogue: dist = sqrt(relu(psq - 2*mx)); partial = sum_n dist.
"""

from contextlib import ExitStack

import ml_dtypes
import numpy as np

import concourse.bass as bass
import concourse.mybir as mybir
import concourse.tile as tile
from concourse import bacc
from concourse.bass import ds
from concourse.bass_utils import run_bass_kernel_spmd

B, N, K, D = 8, 4096, 1024, 256
P = 128
NCORES = 8
MCH = N // P     # 32 row-chunks of 128 points
NPAIR = MCH // 2  # 16 chunk-pairs
GRP = 8          # chunks per tree group
NGRP = MCH // GRP
WG = 4           # weight DMA groups
MPG = MCH // WG

F32 = mybir.dt.float32
BF16 = mybir.dt.bfloat16
FP8 = mybir.dt.float8e4
AF = mybir.ActivationFunctionType
ALU = mybir.AluOpType
DR = mybir.MatmulPerfMode.DoubleRow

# pairs whose PSUM is evacuated by DVE tensor_copy instead of ACT, to
# balance ACT (~1.97us/pair) against DVE tree work (~6us/group); keep them
# mid-group and early so they don't collide with tree work on DVE
DVE_EVAC_PAIRS = frozenset({1, 5})

# group sizes (chunks) for the fused tree; smaller final groups shrink the
# post-pipeline tail (tree of the last group runs after the last evac)
GROUP_SIZES = [8, 8, 8, 4, 4]


def _build_kernel(ctx: ExitStack, tc: tile.TileContext, out, wall, cpack_d, cbar8_d, psqT_d):
    nc = tc.nc

    const_pool = ctx.enter_context(tc.tile_pool(name="const", bufs=1))
    psum_main = ctx.enter_context(tc.tile_pool(name="psum_main", bufs=2, space="PSUM"))
    evp = ctx.enter_context(tc.tile_pool(name="evp", bufs=2))
    trp = ctx.enter_context(tc.tile_pool(name="trp", bufs=2))

    # --- bulk input loads -------------------------------------------------
    cpack = const_pool.tile([P, 2, K], FP8, name="cpack", tag="cpack")
    nc.sync.dma_start(cpack[:], cpack_d[:])

    wt = []
    for g in range(WG):
        w = const_pool.tile([P, MPG, 2, P], FP8, name=f"wt{g}", tag=f"wt{g}")
        eng = nc.sync if g < 2 else nc.gpsimd
        eng.dma_start(w[:], wall[:, ds(g * MPG, MPG), :, :])
        wt.append(w)

    cbar8 = const_pool.tile([P, GRP, P], BF16, name="cbar8", tag="cbar8")
    nc.sync.dma_start(cbar8[:], cbar8_d[:])
    psq = const_pool.tile([P, MCH], F32, name="psq", tag="psq")
    nc.gpsimd.dma_start(psq[:], psqT_d[:])

    onescol = const_pool.tile([P, 1], F32, name="onescol", tag="onescol")
    nc.vector.memset(onescol[:], 1.0)

    mx = const_pool.tile([P, MCH], F32, name="mx", tag="mx")

    # --- PE warm-up: dummy matmuls during the input-DMA window keep the PE
    # clock up so the first real pairs run at full speed
    wz = const_pool.tile([P, 2, P], FP8, name="wz", tag="wz")
    nc.vector.memset(wz[:], 0.0)
    warm = psum_main.tile([P, 2, K], F32, name="warm", tag="pp")
    for i in range(10):
        nc.tensor.matmul(
            warm[:, i % 2, ds(0, P)], wz[:], wz[:],
            start=True, stop=True, perf_mode=DR,
        )

    # --- main loop: 16 chunk-pairs, fused tree per group ------------------
    group_of = []
    group_base = []
    base = 0
    for gi, gs in enumerate(GROUP_SIZES):
        for _ in range(gs):
            group_of.append(gi)
            group_base.append(base)
        base += gs

    ev8 = None
    for t in range(NPAIR):
        m0 = 2 * t
        gi = group_of[m0]
        gs = GROUP_SIZES[gi]
        gb = group_base[m0]
        slot = m0 - gb
        if slot == 0:
            ev8 = evp.tile([P, gs, K], BF16, name="ev8", tag="ev8")

        pp = psum_main.tile([P, 2, K], F32, name="pp", tag="pp")
        for c in range(2):
            m = 2 * t + c
            w = wt[m // MPG][:, m % MPG, :, :]
            for kh in range(K // 512):
                nc.tensor.matmul(
                    pp[:, c, ds(kh * 512, 512)], w, cpack[:, :, ds(kh * 512, 512)],
                    start=True, stop=True, perf_mode=DR,
                )

        evslice = ev8[:, ds(slot, 2), :]
        if t in DVE_EVAC_PAIRS:
            nc.vector.tensor_copy(evslice, pp[:])
        else:
            nc.scalar.copy(evslice, pp[:])

        if slot == gs - 2:
            # fused tree over the whole group
            t1 = trp.tile([P, gs, 512], BF16, name="t1", tag="t1")
            nc.vector.tensor_max(t1[:], ev8[:, :, ds(0, 512)], ev8[:, :, ds(512, 512)])
            t2 = trp.tile([P, gs, 256], BF16, name="t2", tag="t2")
            nc.vector.tensor_max(t2[:], t1[:, :, ds(0, 256)], t1[:, :, ds(256, 256)])
            t3 = trp.tile([P, gs, P], BF16, name="t3", tag="t3")
            nc.vector.tensor_max(t3[:], t2[:, :, ds(0, P)], t2[:, :, ds(P, P)])
            s3 = trp.tile([P, gs, P], BF16, name="s3", tag="s3")
            nc.vector.tensor_sub(s3[:], t3[:], cbar8[:, ds(0, gs), :])
            t4 = trp.tile([P, gs, 64], BF16, name="t4", tag="t4")
            nc.vector.tensor_max(t4[:], s3[:, :, ds(0, 64)], s3[:, :, ds(64, 64)])
            nc.vector.tensor_reduce(
                mx[:, ds(gb, gs)], t4[:], mybir.AxisListType.X, ALU.max
            )

    # --- epilogue: dist = sqrt(relu(psq - 2*mx)); partial = sum dist ------
    d2b = const_pool.tile([P, MCH], F32, name="d2b", tag="d2b")
    nc.vector.scalar_tensor_tensor(d2b[:], mx[:], -2.0, psq[:], ALU.mult, ALU.add)
    d2r = const_pool.tile([P, MCH], F32, name="d2r", tag="d2r")
    nc.vector.tensor_scalar_max(d2r[:], d2b[:], 0.0)
    dist = const_pool.tile([P, MCH], F32, name="dist", tag="dist")
    nc.scalar.activation(dist[:], d2r[:], AF.Sqrt)
    rowsum = const_pool.tile([P, 1], F32, name="rowsum", tag="rowsum")
    nc.vector.tensor_reduce(rowsum[:], dist[:], mybir.AxisListType.X, ALU.add)
    fin = psum_main.tile([1, 1, 1], F32, name="fin", tag="pp", padded_shape=[P, 2, K])
    nc.tensor.matmul(fin[:], rowsum[:], onescol[:], start=True, stop=True)
    out_sb = const_pool.tile([1, 1], F32, name="out_sb", tag="out_sb")
    nc.scalar.copy(out_sb[:], fin[:])
    nc.gpsimd.dma_start(out[:], out_sb[:])


def build():
    nc = bacc.Bacc(
        "TRN2",
        target_bir_lowering=False,
        debug=False,
        enable_asserts=False,
        num_devices=NCORES,
    )
    wall = nc.dram_tensor("wall", [P, MCH, 2, P], FP8, kind="ExternalInput").ap()
    cpack_d = nc.dram_tensor("cpack", [P, 2, K], FP8, kind="ExternalInput").ap()
    cbar8_d = nc.dram_tensor("cbar8", [P, GRP, P], BF16, kind="ExternalInput").ap()
    psqT_d = nc.dram_tensor("psqT", [P, MCH], F32, kind="ExternalInput").ap()
    out = nc.dram_tensor("out", [1, 1], F32, kind="ExternalOutput").ap()
    with tile.TileContext(nc) as tc, ExitStack() as ctx:
        _build_kernel(ctx, tc, out, wall, cpack_d, cbar8_d, psqT_d)
    nc.compile()
    return nc


_NC = None


def _make_in_maps(points: np.ndarray, centers: np.ndarray):
    # column c of cpack holds sorted-rank r = (c % 128)*8 + c//128, so the
    # max-tree block {j, j+128, ..., j+896} covers ranks 8j..8j+7
    cols = np.arange(K)
    rank_of_col = (cols % P) * GRP + cols // P
    in_maps = []
    for b in range(B):
        p8 = points[b].astype(ml_dtypes.float8_e4m3)    # [N, D]
        c8 = centers[b].astype(ml_dtypes.float8_e4m3)   # [K, D]
        pf = p8.astype(np.float32)
        cf = c8.astype(np.float32)
        psq = np.einsum("nd,nd->n", pf, pf)             # [N]
        csqh = 0.5 * np.einsum("kd,kd->k", cf, cf)      # [K]
        order = np.argsort(csqh)
        c8i = c8[order][rank_of_col]                    # centers in column order
        blk = csqh[order].reshape(P, GRP)               # block j = ranks 8j..8j+7
        cbar = 0.5 * (blk.min(1) + blk.max(1))          # [128] midpoint
        cbar8 = np.broadcast_to(
            cbar.astype(ml_dtypes.bfloat16)[None, None, :], (P, GRP, P)
        )
        # wall[p, m, s, n] = p8[m*128+n, s*128+p]
        wall = np.ascontiguousarray(
            p8.reshape(MCH, P, 2, P).transpose(3, 0, 2, 1)
        )
        # cpack[p, s, k] = c8i[k, s*128+p]
        cpack = np.ascontiguousarray(c8i.reshape(K, 2, P).transpose(2, 1, 0))
        in_maps.append(
            {
                "wall": wall,
                "cpack": cpack,
                "cbar8": np.ascontiguousarray(cbar8),
                "psqT": np.ascontiguousarray(psq.reshape(MCH, P).T),
            }
        )
    return in_maps


def kernel(points, centers, **_run_kwargs):
    global _NC
    points = np.asarray(points, dtype=np.float32)
    centers = np.asarray(centers, dtype=np.float32)
    assert points.shape == (B, N, D) and centers.shape == (B, K, D)
    if _NC is None:
        _NC = build()
    res = run_bass_kernel_spmd(
        _NC, _make_in_maps(points, centers), list(range(NCORES)), **_run_kwargs
    )
    total = sum(float(r["out"][0, 0]) for r in res.results)
    return np.array(total / (B * N), dtype=np.float32)


if __name__ == "__main__":
    pts = np.random.RandomState(0).randn(B, N, D).astype(np.float32)
    ctr = np.random.RandomState(1).randn(B, K, D).astype(np.float32)
    print(kernel(pts, ctr))
